# revision 16
# baseline (speedup 1.0000x reference)
"""ESM2 contact predictor head on 8 Trainium2 NeuronCores.

Computes out[b, i, j] = sigmoid(x[b,i] @ W @ x[b,j] + bias) for
x: (8, 2050, 320) f32, W: (320, 320) f32, bias: (1,) f32.

Sharding: data-parallel over batch — core c handles batch element c.

Per-core algorithm (fp16 operands, f32 PSUM accumulation — rel err
~7e-4 vs the f32 reference; fp16 streams at the same 1 col/cycle PE
rate as fp32r but halves LDWEIGHTS and input-DMA time):
  host:  xt[p, k, j] = x[j, 128k+p] as 5 contiguous chunks of 410
         columns (D=320 zero-padded to 384 = 3 K-slabs of 128);
         w = W zero-padded to (384, 384), slab-major.
  chip:  warmup matmuls release the PE clock-gate while inputs stream;
         ph1: u^T[e, i] = sum_d W[d,e] xt[d,i]  (3 e-blocks x 3 K-slabs
              per 410-col chunk, PSUM f32 -> fp16 u_sb via DVE cast)
         ph2: 16 row-strips (M=128) + one M=2 tail strip, each 5
              j-blocks of 410 cols; consecutive matmuls always target
              alternating PSUM banks (same-bank back-to-back matmuls
              stall ~105 ns on the accumulate turnaround), so j-blocks
              are processed in pairs k-outer, and the odd 5th j-block
              is paired across adjacent strips. Fused sigmoid+bias on
              ScalarE (one op per j-block pair) -> contiguous row DMA
              into the (2050, 2050) output.
         ph1 chunks are interleaved between early strips so the PE
         never idles while the input DMAs finish.
"""

import numpy as np

import concourse.mybir as mybir
import concourse.tile as tile
from concourse import bacc
from concourse.bass_utils import run_bass_kernel_spmd

N_CORES = 8
B, L, D = 8, 2050, 320
KT = 3                  # K slabs (zero-padded to 3 x 128)
NCH, CW = 5, 410        # j/i chunking: 5 x 410 = 2050 exactly
F32 = mybir.dt.float32
F16 = mybir.dt.float16
SIG = mybir.ActivationFunctionType.Sigmoid
NWARM = 4

_cache = {}


def _build(bias_val: float):
    nc = bacc.Bacc("TRN2", target_bir_lowering=False, debug=False,
                   num_devices=N_CORES)
    xt_d = nc.dram_tensor("xt", [NCH, KT, 128, CW], F16, kind="ExternalInput")
    w_d = nc.dram_tensor("w", [128, KT, 384], F16, kind="ExternalInput")
    out_d = nc.dram_tensor("out", [L, L], F16, kind="ExternalOutput")

    with tile.TileContext(nc) as tc:
        with (
            tc.tile_pool(name="persist", bufs=1) as pp,
            tc.tile_pool(name="outp", bufs=4) as outp,
            tc.tile_pool(name="psum", bufs=4, space="PSUM") as psp,
        ):
            bias_t = pp.tile([128, 1], F32)
            nc.vector.memset(bias_t[:], bias_val)

            w_sb = pp.tile([128, KT, 384], F16)
            xt_sb = pp.tile([128, KT, NCH, CW], F16)
            u_sb = pp.tile([128, KT, L], F16)

            # W kicks go on the Activation queue so they issue in parallel
            # with the Sync queue's chunk kicks (each kick costs ~600 ns of
            # queue time); chunk 0 is split per K-slab so ph1(0) can start
            # as soon as its first pieces land.
            nc.scalar.dma_start(w_sb[:], w_d.ap())
            for k in range(KT):
                nc.sync.dma_start(xt_sb[:, k, 4, :], xt_d.ap()[4][k])
            for k in range(KT):
                nc.sync.dma_start(xt_sb[:, k, 0, :], xt_d.ap()[0][k])
            for k in range(KT):
                nc.scalar.dma_start(xt_sb[:, k, 1, :], xt_d.ap()[1][k])
            for k in range(KT):
                nc.sync.dma_start(xt_sb[:, k, 2, :], xt_d.ap()[2][k])
            for k in range(KT):
                nc.scalar.dma_start(xt_sb[:, k, 3, :], xt_d.ap()[3][k])

            # PE warmup: dummy matmuls release the HAM clock-gate and burn
            # the p-state ramp while the input DMAs land.
            warm_sb = pp.tile([128, 512], F16)
            nc.vector.memset(warm_sb[:], 0.0)
            psw = psp.tile([128, 2, 512], F32, tag="ps", name="psw")
            for wi in range(NWARM):
                nc.tensor.matmul(psw[:, wi % 2, :], lhsT=warm_sb[:, :128],
                                 rhs=warm_sb[:], start=True, stop=True)
            # preload the sigmoid activation table while DMAs run
            act_warm = pp.tile([128, 1], F32)
            nc.scalar.activation(act_warm[:], bias_t[:], SIG)

            def mm_pair(lhs_of, rhs_a, rhs_b, ps, m):
                # 6 matmuls, k-outer, alternating the two PSUM banks so no
                # two consecutive matmuls hit the same bank.
                for k in range(KT):
                    u_k = lhs_of(k)
                    nc.tensor.matmul(ps[:m, 0, 0:CW], lhsT=u_k,
                                     rhs=rhs_a(k), start=(k == 0),
                                     stop=(k == KT - 1))
                    nc.tensor.matmul(ps[:m, 1, 0:CW], lhsT=u_k,
                                     rhs=rhs_b(k), start=(k == 0),
                                     stop=(k == KT - 1))

            tail_t = pp.tile([128, NCH, CW], F16)

            def ph1_full(c, tail=False):
                # u^T[e, 410c:410c+410] for all 3 e-blocks from xt chunk c;
                # the 9 matmuls rotate over 3 PSUM banks so no consecutive
                # pair hits the same bank.
                def wl(et):
                    return lambda k: w_sb[:, k, 128 * et:128 * et + 128]

                def xr(k, _c=c):
                    return xt_sb[:, k, _c, :]

                ps = psp.tile([128, 2, 512], F32, tag="ps", name="ph1p")
                ps2 = psp.tile([128, 2, 512], F32, tag="ps", name="ph1s")
                for k in range(KT):
                    nc.tensor.matmul(ps[:, 0, 0:CW], lhsT=wl(0)(k), rhs=xr(k),
                                     start=(k == 0), stop=(k == KT - 1))
                    nc.tensor.matmul(ps[:, 1, 0:CW], lhsT=wl(1)(k), rhs=xr(k),
                                     start=(k == 0), stop=(k == KT - 1))
                    nc.tensor.matmul(ps2[:, 0, 0:CW],
                                     lhsT=w_sb[:, k, 256:384], rhs=xr(k),
                                     start=(k == 0), stop=(k == KT - 1))
                nc.vector.tensor_copy(u_sb[:, 0, CW * c:CW * (c + 1)],
                                      ps[:, 0, 0:CW])
                nc.vector.tensor_copy(u_sb[:, 1, CW * c:CW * (c + 1)],
                                      ps[:, 1, 0:CW])
                nc.vector.tensor_copy(u_sb[:, 2, CW * c:CW * (c + 1)],
                                      ps2[:, 0, 0:CW])
                if tail:
                    # w columns 320:322 were patched with u[:, 2048:2050],
                    # so psum partitions 64:66 are the tail rows' logits.
                    nc.scalar.activation(tail_t[64:66, c, :],
                                         ps2[64:66, 0, 0:CW], SIG,
                                         bias=bias_t[64:66, :])

            def patch_tail_u():
                # requires ph1(4) complete (u columns 2048:2050 written)
                for k in range(KT):
                    nc.vector.tensor_copy(w_sb[:, k, 320:322],
                                          u_sb[:, k, 2048:2050])

            def tail_chunk4():
                # tail rows x chunk-4 columns: the one piece the patched
                # ph1_et2 passes don't cover. M=2 matmul, then DMA the
                # whole (2, 2050) tail out mid-kernel.
                ps = psp.tile([128, 2, 512], F32, tag="ps", name="pst")
                for k in range(KT):
                    nc.tensor.matmul(ps[:2, 0, 0:CW],
                                     lhsT=u_sb[:, k, 2048:2050],
                                     rhs=xt_sb[:, k, 4, :],
                                     start=(k == 0), stop=(k == KT - 1))
                nc.scalar.activation(tail_t[0:2, 4, :], ps[:2, 0, 0:CW], SIG,
                                     bias=bias_t[0:2, :])
                nc.scalar.dma_start(out_d.ap()[2048:2050, 0:1640],
                                    tail_t[64:66, 0:4, :])
                nc.scalar.dma_start(out_d.ap()[2048:2050, 1640:2050],
                                    tail_t[0:2, 4, :])

            strip_out = {}

            def strip_main(i0, m, fine=False):
                # j-blocks 0..3 of one output row-strip (rows i0:i0+m)
                outt = outp.tile([128, NCH, CW], F16, tag="strip", bufs=8,
                                 name="outt")
                strip_out[i0] = outt

                def ul(k):
                    return u_sb[:, k, i0:i0 + m]

                for half in range(2):
                    ps = psp.tile([128, 2, 512], F32, tag="ps", name="ps")
                    mm_pair(ul,
                            lambda k, c=2 * half: xt_sb[:, k, c, :],
                            lambda k, c=2 * half + 1: xt_sb[:, k, c, :],
                            ps, m)
                    nc.scalar.activation(outt[:m, 2 * half:2 * half + 2, :],
                                         ps[:m, :, 0:CW], SIG,
                                         bias=bias_t[:m, :])
                    if fine:
                        nc.sync.dma_start(
                            out_d.ap()[i0:i0 + m, 820 * half:820 * half + 820],
                            outt[:m, 2 * half:2 * half + 2, :])
                if not fine:
                    nc.sync.dma_start(out_d.ap()[i0:i0 + m, 0:1640],
                                      outt[:m, 0:4, :])

            def strip_j4(i0a, m_a, i0b, m_b):
                # the odd 5th j-block (cols 1640:2050) for two row-strips,
                # paired so consecutive matmuls alternate PSUM banks.
                ps = psp.tile([128, 2, 512], F32, tag="ps", name="ps4")
                for k in range(KT):
                    nc.tensor.matmul(ps[:m_a, 0, 0:CW],
                                     lhsT=u_sb[:, k, i0a:i0a + m_a],
                                     rhs=xt_sb[:, k, 4, :],
                                     start=(k == 0), stop=(k == KT - 1))
                    nc.tensor.matmul(ps[:m_b, 1, 0:CW],
                                     lhsT=u_sb[:, k, i0b:i0b + m_b],
                                     rhs=xt_sb[:, k, 4, :],
                                     start=(k == 0), stop=(k == KT - 1))
                for sub, (i0, m) in enumerate(((i0a, m_a), (i0b, m_b))):
                    outt = strip_out.pop(i0)
                    nc.scalar.activation(outt[:m, 4, :], ps[:m, sub, 0:CW],
                                         SIG, bias=bias_t[:m, :])
                    nc.scalar.dma_start(out_d.ap()[i0:i0 + m, 1640:2050],
                                        outt[:m, 4, :])

            def strip_j4_solo(i0, m):
                ps = psp.tile([128, 2, 512], F32, tag="ps", name="ps4")
                for k in range(KT):
                    nc.tensor.matmul(ps[:m, 0, 0:CW],
                                     lhsT=u_sb[:, k, i0:i0 + m],
                                     rhs=xt_sb[:, k, 4, :],
                                     start=(k == 0), stop=(k == KT - 1))
                outt = strip_out.pop(i0)
                nc.scalar.activation(outt[:m, 4, :], ps[:m, 0, 0:CW], SIG,
                                     bias=bias_t[:m, :])
                nc.sync.dma_start(out_d.ap()[i0:i0 + m, :], outt[:m])

            # ph1(4) first (chunk 4 arrives first), so u[:, 2048:2050]
            # exists and can be patched into w's zero-pad columns; the
            # remaining ph1 chunks then compute the tail rows for free.
            ph1_full(4)
            patch_tail_u()
            ph1_full(0, tail=True)
            ph1_full(1, tail=True)
            strip_main(0, 128)
            ph1_full(2, tail=True)
            strip_main(128, 128)
            strip_j4(0, 128, 128, 128)
            ph1_full(3, tail=True)
            tail_chunk4()
            strip_main(256, 128)
            strip_main(384, 128)
            strip_j4(256, 128, 384, 128)
            for sp in range(2, 8):
                a, b = 2 * sp, 2 * sp + 1
                strip_main(128 * a, 128)
                strip_main(128 * b, 128, fine=(sp == 7))
                strip_j4(128 * a, 128, 128 * b, 128)

    nc.compile()
    return nc


last_results = None


def _host_pack(x, W):
    xT = x.transpose(0, 2, 1)  # (B, 320, L)
    full = np.zeros((B, 128, KT, L), np.float16)
    full[:, :, 0, :] = xT[:, 0:128]
    full[:, :, 1, :] = xT[:, 128:256]
    full[:, 0:64, 2, :] = xT[:, 256:320]
    xt_in = np.ascontiguousarray(
        full.reshape(B, 128, KT, NCH, CW).transpose(0, 3, 2, 1, 4))
    Wp = np.zeros((384, 384), np.float16)
    Wp[0:320, 0:320] = W.astype(np.float16)
    w_in = np.ascontiguousarray(Wp.reshape(KT, 128, 384).transpose(1, 0, 2))
    return xt_in, w_in


def kernel(x, W, b, _trace=False):
    global last_results
    x = np.ascontiguousarray(np.asarray(x, dtype=np.float32))
    W = np.asarray(W, dtype=np.float32)
    b = np.asarray(b, dtype=np.float32)
    bias_val = float(b[0])

    if bias_val not in _cache:
        _cache.clear()
        _cache[bias_val] = _build(bias_val)
    nc = _cache[bias_val]

    xt_in, w_in = _host_pack(x, W)
    in_maps = [{"xt": xt_in[c], "w": w_in} for c in range(N_CORES)]
    res = run_bass_kernel_spmd(nc, in_maps, core_ids=list(range(N_CORES)),
                               trace=_trace)
    last_results = res
    out = np.empty((B, L, L), dtype=np.float32)
    for c in range(N_CORES):
        out[c] = res.results[c]["out"].astype(np.float32)
    return out
